# revision 27
# baseline (speedup 1.0000x reference)
"""Trainium2 Bass kernel for the per-cell star-graph GAT encoder.

Math: the reference returns only the anchor-node (node 0) output of a 1-layer
GAT over a (T+1)-node graph per cell. The anchor's adjacency row contains only
the star edges (anchor self-loop + all valid transcripts), so the kNN graph is
dead code for the output. With h_anchor = 0 the output reduces, per cell, to

    s_j    = tx_feat_j . (W_gat @ a_src)            (per transcript scalar)
    l_j    = leaky_relu(s_j, 0.2)  (+ -1e30 where invalid)
    e_j    = exp(l_j)        den = 1 + sum_j e_j    (anchor logit is 0)
    pooled = sum_j e_j tx_feat_j / den              (35-dim)
    out    = pooled @ W_gat + b_gat, zeroed where cell inactive

where tx_feat_j = [rel_xy (2) | gene_emb[id_j] (32) | qv (1)].

Sharding: data-parallel over the 1024 = B*Q cells, 128 cells per NeuronCore.
Per core, cells sit on SBUF partitions; the gene table (with the per-gene
scalar gs = gene_emb @ wa[2:34] prepended as column 0, padded to 256B rows)
is gathered from DRAM by dma_gather in tapering t-chunks of 4 quarter ops
each, spread over the 4 SWDGE queues (pattern 1,2,3,0: three async queues
dispatch instantly, queue 0 holds the engine while all four generate
descriptors concurrently), pipelined against the DVE/ACT compute. The gather
phase is Q7 descriptor-generation bound (~8.4 ns/idx/queue, 4-queue cap).

All dense inputs ride in ONE packed [128, NB] f32 tensor -> a single input
DMA. The epilogue matmul runs in bf16 (W packed bf16 on host, lhsT cast via
the ACT copy out of the PE-transpose PSUM) and is split: chunks 0..NCH-3 feed
an early matmul into PSUM (start=True) overlapped with the late gathers; the
last two chunks' contribution accumulates into the same bank so only a small
transpose+matmul+scale chain trails the final gather. Output is bf16, widened
on host (rel err ~4e-3 vs the 2e-2 gate).
"""

import numpy as np

import concourse.bacc as bacc
import concourse.mybir as mybir
import concourse.tile as tile
from concourse.bass_utils import run_bass_kernel_spmd
from concourse.tile import add_dep_helper

F32 = mybir.dt.float32
BF16 = mybir.dt.bfloat16
I16 = mybir.dt.int16
AX = mybir.AxisListType
OP = mybir.AluOpType
AF = mybir.ActivationFunctionType

N_CORES = 8
B, Q, T = 4, 256, 128
CELLS = (B * Q) // N_CORES  # 128 cells per core
GENE_VOCAB = 20000
GENE_DIM = 32
F = 64  # padded table row: [gs | gene_emb(32) | zeros], 256B for dma_gather
IN_DIM = 35
D = 256
NEG_SLOPE = 0.2
CHUNK_T = (48, 32, 24, 20, 4)  # big early, tiny tail round
NCH = len(CHUNK_T)
SINGLE_PACKET = False
assert sum(CHUNK_T) == T
NLATE = 2                # chunks NCH-NLATE.. land after the bulk matmul
TM = sum(CHUNK_T[:NCH - NLATE])  # early-transcript count (pooled_p x/qv)

# packed input column layout (f32 columns)
C_X = 0            # x: T*2
C_QV = C_X + T * 2       # 256
C_MADD = C_QV + T        # 384
C_CENT = C_MADD + T      # 512
C_ACT = C_CENT + 2       # 514
C_WAT = C_ACT + 1        # 515: wa01 tiled, 256
C_BG = C_WAT + 256       # 771: b_gat, 256
C_WA34 = C_BG + 256      # 1027
C_ID = C_WA34 + 1        # 1028: identity 128
C_WR = C_ID + 128        # 1156: wrhs bf16 [128, 256] as 128 f32
C_IDX = C_WR + 128       # 1284: idx int16 [128, 1024] as 512 f32
NB = C_IDX + (CELLS * T) // 32  # 1796

_CACHE = {}


def build_program():
    nc = bacc.Bacc("TRN2", target_bir_lowering=False, debug=False,
                   num_devices=N_CORES, num_swdge_queues=4)

    big_d = nc.dram_tensor("big_in", [CELLS, NB], F32, kind="ExternalInput")
    table_d = nc.dram_tensor("table_in", [GENE_VOCAB, F], F32, kind="ExternalInput")
    out_d = nc.dram_tensor("out", [CELLS, D], BF16, kind="ExternalOutput")

    from concourse import library_config

    with tile.TileContext(nc) as tc:
        with (
            tc.tile_pool(name="single", bufs=1) as single,
            tc.tile_pool(name="gpool", bufs=NCH) as gpool,
            tc.tile_pool(name="work", bufs=2) as work,
            tc.tile_pool(name="stats", bufs=NCH + 2) as stats,
            tc.tile_pool(name="psum", bufs=2, space="PSUM") as psum,
        ):
            # the mlp-library Q7 IRAM reload (~11us) gates the first gather;
            # it must be the first thing on the gpsimd stream
            nc.gpsimd.load_library(library_config.mlp)

            # one packed DMA for every dense input (sync HWDGE ring)
            big = single.tile([CELLS, NB], F32)
            nc.sync.dma_start(out=big[:], in_=big_d.ap())
            x_sb = big[:, C_X:C_X + T * 2]
            qv_sb = big[:, C_QV:C_QV + T]
            madd_sb = big[:, C_MADD:C_MADD + T]
            cent_sb = big[:, C_CENT:C_CENT + 2]
            act_sb = big[:, C_ACT:C_ACT + 1]
            wat_sb = big[:, C_WAT:C_WAT + 256]
            bg_sb = big[:, C_BG:C_BG + 256]
            wa34_sb = big[:, C_WA34:C_WA34 + 1]
            ident = big[:, C_ID:C_ID + 128]
            wrhs_sb = big[:IN_DIM, C_WR:C_WR + 128].bitcast(BF16)  # [35, 256]
            idx_sb = big[:, C_IDX:NB].bitcast(I16)  # [128, 1024] int16

            # --- gather pipeline (critical path) ---
            # queue pattern [1,2,3,0] per chunk: the three async queues
            # dispatch instantly, then the sync queue-0 op holds the engine
            # while all four queues' desc-gen runs concurrently. The
            # period-4 pattern also keeps Tile's 8 round-robin DMA-sem lanes
            # queue-consistent.
            g_t = []
            t_base = 0
            for j in range(NCH):
                tcnt = CHUNK_T[j]
                tq = tcnt // 4  # transcripts per quarter-gather
                gj = gpool.tile([CELLS, max(CHUNK_T) * F], F32, tag="g")
                g3w = gj[:].rearrange("p (t f) -> p t f", t=max(CHUNK_T), f=F)
                for sub, queue in enumerate((1, 2, 3, 0)):
                    c0 = (CELLS * (t_base + tq * sub)) // 16
                    nc.gpsimd.dma_gather(
                        out_ap=g3w[:, tq * sub:tq * (sub + 1), :],
                        in_ap=table_d.ap(),
                        idxs_ap=idx_sb[:, c0:c0 + (CELLS * tq) // 16],
                        num_idxs=CELLS * tq,
                        num_idxs_reg=CELLS * tq,
                        elem_size=F,
                        single_packet=SINGLE_PACKET,
                        queue_num=queue,
                    )
                g_t.append(gj)
                t_base += tcnt

            # b_gat * active, ready off the critical path for the epilogue
            bact = single.tile([CELLS, D], F32)
            nc.vector.tensor_scalar_mul(bact[:], bg_sb, act_sb)

            # --- s_pre = (rel . wa01) + qv*wa34 - cent.wa01 + madd  (no gene) ---
            xw = work.tile([CELLS, T * 2], F32)
            nc.vector.tensor_tensor(out=xw[:], in0=x_sb, in1=wat_sb, op=OP.mult)
            term_x = single.tile([CELLS, T], F32)
            nc.vector.tensor_reduce(
                out=term_x[:],
                in_=xw[:].rearrange("p (t c) -> p t c", t=T, c=2),
                axis=AX.X, op=OP.add)
            cw = stats.tile([CELLS, 2], F32, tag="cw")
            nc.vector.tensor_tensor(out=cw[:], in0=cent_sb, in1=wat_sb[:, 0:2],
                                    op=OP.mult)
            c01 = stats.tile([CELLS, 1], F32, tag="c01")
            nc.vector.tensor_reduce(out=c01[:], in_=cw[:], axis=AX.X, op=OP.add)
            negc01 = stats.tile([CELLS, 1], F32, tag="negc01")
            nc.vector.tensor_scalar_mul(negc01[:], c01[:], -1.0)
            qvs = work.tile([CELLS, T], F32, tag="qvs")
            nc.scalar.activation(out=qvs[:], in_=qv_sb, func=AF.Identity,
                                 bias=negc01[:], scale=wa34_sb)
            spre0 = work.tile([CELLS, T], F32, tag="spre0")
            nc.vector.tensor_tensor(out=spre0[:], in0=term_x[:], in1=qvs[:], op=OP.add)
            spre = single.tile([CELLS, T], F32)
            nc.vector.tensor_tensor(out=spre[:], in0=spre0[:], in1=madd_sb, op=OP.add)

            # --- per-chunk: s -> leaky relu -> exp -> weighted gene partials.
            # exp writes straight into e_all. Chunks 0..NCH-NLATE-1 feed
            # pooled_p -> an EARLY matmul into PSUM (start=True) that runs
            # under the late chunks' gathers; the NLATE late chunks feed
            # pooled_l, accumulated into the same bank after the last gather.
            e_all = single.tile([CELLS, T], F32, name="e_all", tag="e_all")
            pooled_p = single.tile([CELLS, IN_DIM], F32)  # early chunks
            pooled_l = single.tile([CELLS, IN_DIM], F32)  # late chunks + corr
            acc_es = None   # [CELLS, 1] running sum of e (early chunks)
            acc_pg = None   # [CELLS, 32] running gene pool (early chunks)
            late_es = []    # [CELLS, 1] per late chunk
            late_pg = []    # [CELLS, 32] per late chunk
            prev_pg_inst = None
            NEARLY = NCH - NLATE
            t_base = 0
            for j in range(NCH):
                tcnt = CHUNK_T[j]
                cj = slice(t_base, t_base + tcnt)
                g3 = g_t[j][:].rearrange("p (t f) -> p t f", t=max(CHUNK_T),
                                         f=F)[:, :tcnt, :]
                s_j = work.tile([CELLS, tcnt], F32, tag="s")
                s_inst = nc.vector.tensor_tensor(out=s_j[:], in0=spre[:, cj],
                                                 in1=g3[:, :, 0:1], op=OP.add)
                if prev_pg_inst is not None:
                    # keep the DVE stream in chunk order: without this the
                    # scheduler (using the serialized-gather cost model) parks
                    # every eg/pg op after the last chunk's s/l
                    add_dep_helper(s_inst.ins, prev_pg_inst.ins, False,
                                   "chunk-order DVE stream")
                l_j = work.tile([CELLS, tcnt], F32, tag="l")
                nc.vector.scalar_tensor_tensor(out=l_j[:], in0=s_j[:],
                                               scalar=NEG_SLOPE, in1=s_j[:],
                                               op0=OP.mult, op1=OP.max)
                esum_j = stats.tile([CELLS, 1], F32, tag="esum")
                nc.scalar.activation(out=e_all[:, cj], in_=l_j[:], func=AF.Exp,
                                     accum_out=esum_j[:])
                eg_j = work.tile([CELLS, tcnt * GENE_DIM], F32, tag="eg")
                nc.vector.tensor_tensor(
                    out=eg_j[:], in0=g3[:, :, 1:1 + GENE_DIM],
                    in1=e_all[:, cj].to_broadcast([CELLS, tcnt, GENE_DIM]),
                    op=OP.mult)
                pg_j = stats.tile([CELLS, GENE_DIM], F32, tag="pg")
                prev_pg_inst = nc.vector.tensor_reduce(
                    out=pg_j[:],
                    in_=eg_j[:].rearrange("p (t f) -> p f t", t=tcnt,
                                          f=GENE_DIM),
                    axis=AX.X, op=OP.add)
                if j >= NEARLY:
                    late_es.append(esum_j)
                    late_pg.append(pg_j)
                    if j == NEARLY:
                        # partial den = acc_es + es_late0 + 1, hidden under
                        # the last chunk's gather
                        es_l = stats.tile([CELLS, 1], F32, tag="esl")
                        nc.vector.tensor_scalar(es_l[:], acc_es[:],
                                                esum_j[:], 1.0, OP.add, OP.add)
                elif acc_es is None:
                    acc_es, acc_pg = esum_j, pg_j
                elif j < NEARLY - 1:
                    new_es = stats.tile([CELLS, 1], F32, tag="aes")
                    nc.vector.tensor_tensor(out=new_es[:], in0=acc_es[:],
                                            in1=esum_j[:], op=OP.add)
                    new_pg = stats.tile([CELLS, GENE_DIM], F32, tag="apg")
                    nc.vector.tensor_tensor(out=new_pg[:], in0=acc_pg[:],
                                            in1=pg_j[:], op=OP.add)
                    acc_es, acc_pg = new_es, new_pg
                else:
                    # final early accumulate writes straight into pooled_p
                    new_es = stats.tile([CELLS, 1], F32, tag="aes")
                    nc.vector.tensor_tensor(out=new_es[:], in0=acc_es[:],
                                            in1=esum_j[:], op=OP.add)
                    acc_es = new_es
                    nc.vector.tensor_tensor(out=pooled_p[:, 2:2 + GENE_DIM],
                                            in0=acc_pg[:], in1=pg_j[:],
                                            op=OP.add)
                t_base += tcnt
                if j == NEARLY - 1:
                    # batched pooled-x / pooled-qv over the early chunks,
                    # straight into pooled_p (runs under the late chunks'
                    # gathers), feeding the early half-matmul into PSUM
                    ex = work.tile([CELLS, TM * 2], F32, tag="exall")
                    nc.vector.tensor_tensor(
                        out=ex[:], in0=x_sb[:, :TM * 2],
                        in1=e_all[:, :TM].to_broadcast([CELLS, TM, 2]),
                        op=OP.mult)
                    nc.vector.tensor_reduce(
                        out=pooled_p[:, 0:2],
                        in_=ex[:].rearrange("p (t c) -> p c t", t=TM, c=2),
                        axis=AX.X, op=OP.add)
                    pqs_m = work.tile([CELLS, TM], F32, tag="pqm")
                    nc.vector.tensor_tensor(out=pqs_m[:], in0=qv_sb[:, :TM],
                                            in1=e_all[:, :TM], op=OP.mult)
                    nc.vector.tensor_reduce(out=pooled_p[:, 34:35],
                                            in_=pqs_m[:], axis=AX.X, op=OP.add)

            # early half-matmul into PSUM over the early chunks; emitted
            # after the loop so the ACT-queue copy cannot block the late
            # chunks' exp (engine queues are in emission order)
            psum_t1 = psum.tile([128, 128], F32, tag="pt")
            nc.tensor.transpose(out=psum_t1[:IN_DIM, :CELLS],
                                in_=pooled_p[:], identity=ident)
            lhsT1 = single.tile([128, CELLS], BF16)
            nc.scalar.copy(lhsT1[:IN_DIM, :], psum_t1[:IN_DIM, :CELLS])
            out_ps = psum.tile([128, D], F32, tag="out")
            nc.tensor.matmul(out=out_ps[:], lhsT=lhsT1[:IN_DIM, :],
                             rhs=wrhs_sb, start=True, stop=False)

            # --- late-chunk combine: pooled-x/qv (needs only e), gene pool ---
            TL = T - TM
            exl = stats.tile([CELLS, TL * 2], F32, tag="exl")
            nc.vector.tensor_tensor(
                out=exl[:], in0=x_sb[:, TM * 2:],
                in1=e_all[:, TM:].to_broadcast([CELLS, TL, 2]),
                op=OP.mult)
            pqs_l = stats.tile([CELLS, TL], F32, tag="pql")
            nc.vector.tensor_tensor(out=pqs_l[:], in0=qv_sb[:, TM:],
                                    in1=e_all[:, TM:], op=OP.mult)
            px_l = stats.tile([CELLS, 2], F32, tag="pxl")
            nc.vector.tensor_reduce(
                out=px_l[:], in_=exl[:].rearrange("p (t c) -> p c t", t=TL, c=2),
                axis=AX.X, op=OP.add)
            nc.vector.tensor_reduce(out=pooled_l[:, 34:35], in_=pqs_l[:],
                                    axis=AX.X, op=OP.add)
            nc.vector.tensor_tensor(out=pooled_l[:, 2:2 + GENE_DIM],
                                    in0=late_pg[0][:], in1=late_pg[1][:],
                                    op=OP.add)

            # den = (acc_es + es_late0 + 1) + es_late1
            den = stats.tile([CELLS, 1], F32, tag="den")
            nc.vector.tensor_tensor(out=den[:], in0=es_l[:],
                                    in1=late_es[1][:], op=OP.add)
            rec = stats.tile([CELLS, 1], F32, tag="rec")
            nc.vector.reciprocal(rec[:], den[:])
            ra = stats.tile([CELLS, 1], F32, tag="ra")
            nc.vector.tensor_scalar_mul(ra[:], rec[:], act_sb)

            # cs = cent * (den - 1) = cent*den - cent, one fused op;
            # pooled_l[0:2] = px_l - cs  (pooled stays UNNORMALIZED; 1/den
            # folds into the epilogue scale)
            cs = stats.tile([CELLS, 2], F32, tag="cs")
            nc.vector.scalar_tensor_tensor(out=cs[:], in0=cent_sb,
                                           scalar=den[:], in1=cent_sb,
                                           op0=OP.mult, op1=OP.subtract)
            nc.vector.tensor_tensor(out=pooled_l[:, 0:2], in0=px_l[:],
                                    in1=cs[:], op=OP.subtract)

            psum_t2 = psum.tile([128, 128], F32, tag="pt")
            nc.tensor.transpose(out=psum_t2[:IN_DIM, :CELLS], in_=pooled_l[:],
                                identity=ident)
            lhsT2 = single.tile([128, CELLS], BF16)
            nc.scalar.copy(lhsT2[:IN_DIM, :], psum_t2[:IN_DIM, :CELLS])
            nc.tensor.matmul(out=out_ps[:], lhsT=lhsT2[:IN_DIM, :],
                             rhs=wrhs_sb, start=False, stop=True)
            out_sb = work.tile([CELLS, D], BF16, tag="outs")
            nc.vector.scalar_tensor_tensor(out=out_sb[:], in0=out_ps[:],
                                           scalar=ra[:], in1=bact[:],
                                           op0=OP.mult, op1=OP.add)
            nc.sync.dma_start(out=out_d.ap(), in_=out_sb[:])

    nc.compile()
    return nc


def host_prep(omics_x, centroids, omics_gene_ids, omics_qv, omics_valid_mask,
              query_valid_mask, gene_emb, W_gat, a_src, a_dst, b_gat):
    import ml_dtypes
    f32 = np.float32
    wa = (W_gat.astype(np.float64) @ a_src.astype(np.float64)).astype(f32)  # [35]
    gs = (gene_emb.astype(f32) @ wa[2:2 + GENE_DIM]).astype(f32)  # [VOCAB]
    table = np.zeros((GENE_VOCAB, F), f32)  # [VOCAB, 64] (256B rows)
    table[:, 0] = gs
    table[:, 1:1 + GENE_DIM] = gene_emb.astype(f32)

    NC_TOT = B * Q
    x = omics_x.astype(f32).reshape(NC_TOT, T * 2)
    qv = omics_qv.astype(f32).reshape(NC_TOT, T)
    ids = omics_gene_ids.astype(np.int16).reshape(NC_TOT, T)
    cent = centroids.astype(f32).reshape(NC_TOT, 2)
    validf = omics_valid_mask.reshape(NC_TOT, T).astype(f32)
    madd = (validf - 1.0) * f32(1e30)
    active = (query_valid_mask.reshape(NC_TOT).astype(bool)
              & omics_valid_mask.reshape(NC_TOT, T).astype(bool).any(-1))
    active = active.astype(f32)

    wrhs_pad = np.zeros((128, D), f32)
    wrhs_pad[:IN_DIM] = W_gat.astype(f32)
    wrhs_bf = np.ascontiguousarray(
        wrhs_pad.astype(ml_dtypes.bfloat16)).view(np.uint16)  # [128, 256] u16

    in_maps = []
    for c in range(N_CORES):
        sl = slice(c * CELLS, (c + 1) * CELLS)
        big = np.zeros((CELLS, NB), f32)
        big[:, C_X:C_X + T * 2] = x[sl]
        big[:, C_QV:C_QV + T] = qv[sl]
        big[:, C_MADD:C_MADD + T] = madd[sl]
        big[:, C_CENT:C_CENT + 2] = cent[sl]
        big[:, C_ACT] = active[sl]
        big[:, C_WAT:C_WAT + 256] = np.tile(wa[0:2], T)[None, :]
        big[:, C_BG:C_BG + 256] = b_gat.astype(f32)[None, :]
        big[:, C_WA34] = wa[34]
        big[:, C_ID:C_ID + 128] = np.eye(128, dtype=f32)
        big[:, C_WR:C_WR + 128] = wrhs_bf.view(f32)
        # flat gather index i = t*CELLS + cell -> dst[cell, t]; wrapped
        # [128, T*CELLS/16] int16 (idx list tiled down the 8 core slabs)
        flat = ids[sl].T.reshape(-1)
        wrapped = np.ascontiguousarray(
            np.tile(flat.reshape(-1, 16).T, (8, 1)))  # [128, 1024] i16
        big[:, C_IDX:NB] = wrapped.reshape(128, -1).view(f32)
        in_maps.append({"big_in": np.ascontiguousarray(big), "table_in": table})
    return in_maps


def _get_program():
    # the program is fully parameter-independent: one compile, ever
    if "prog" not in _CACHE:
        _CACHE["prog"] = build_program()
    return _CACHE["prog"]


def kernel(omics_x, centroids, omics_gene_ids, omics_qv, omics_valid_mask,
           query_valid_mask, gene_emb, W_gat, a_src, a_dst, b_gat,
           trace=False):
    in_maps = host_prep(
        np.asarray(omics_x), np.asarray(centroids), np.asarray(omics_gene_ids),
        np.asarray(omics_qv), np.asarray(omics_valid_mask),
        np.asarray(query_valid_mask), np.asarray(gene_emb), np.asarray(W_gat),
        np.asarray(a_src), np.asarray(a_dst), np.asarray(b_gat))
    nc = _get_program()
    res = run_bass_kernel_spmd(nc, in_maps, core_ids=list(range(N_CORES)),
                               trace=trace)
    global LAST_RESULTS
    LAST_RESULTS = res
    outs = [np.asarray(res.results[c]["out"]).astype(np.float32)
            for c in range(N_CORES)]
    full = np.concatenate(outs, axis=0).reshape(B, Q, D)
    return full.astype(np.float32)


# revision 28
# speedup vs baseline: 1.0179x; 1.0179x over previous
"""Trainium2 Bass kernel for the per-cell star-graph GAT encoder.

Math: the reference returns only the anchor-node (node 0) output of a 1-layer
GAT over a (T+1)-node graph per cell. The anchor's adjacency row contains only
the star edges (anchor self-loop + all valid transcripts), so the kNN graph is
dead code for the output. With h_anchor = 0 the output reduces, per cell, to

    s_j    = tx_feat_j . (W_gat @ a_src)            (per transcript scalar)
    l_j    = leaky_relu(s_j, 0.2)  (+ -1e30 where invalid)
    e_j    = exp(l_j)        den = 1 + sum_j e_j    (anchor logit is 0)
    pooled = sum_j e_j tx_feat_j / den              (35-dim)
    out    = pooled @ W_gat + b_gat, zeroed where cell inactive

where tx_feat_j = [rel_xy (2) | gene_emb[id_j] (32) | qv (1)].

Sharding: data-parallel over the 1024 = B*Q cells, 128 cells per NeuronCore.
Per core, cells sit on SBUF partitions; the gene table (with the per-gene
scalar gs = gene_emb @ wa[2:34] prepended as column 0, padded to 256B rows)
is gathered from DRAM by dma_gather in tapering t-chunks of 4 quarter ops
each, spread over the 4 SWDGE queues (pattern 1,2,3,0: three async queues
dispatch instantly, queue 0 holds the engine while all four generate
descriptors concurrently), pipelined against the DVE/ACT compute. The gather
phase is Q7 descriptor-generation bound (~8.4 ns/idx/queue, 4-queue cap).

All dense inputs ride in ONE packed [128, NB] f32 tensor -> a single input
DMA. The epilogue matmul runs in bf16 (W packed bf16 on host, lhsT cast via
the ACT copy out of the PE-transpose PSUM) and is split: chunks 0..NCH-3 feed
an early matmul into PSUM (start=True) overlapped with the late gathers; the
last two chunks' contribution accumulates into the same bank so only a small
transpose+matmul+scale chain trails the final gather. Output is bf16, widened
on host (rel err ~4e-3 vs the 2e-2 gate).
"""

import numpy as np

import concourse.bacc as bacc
import concourse.mybir as mybir
import concourse.tile as tile
from concourse.bass_utils import run_bass_kernel_spmd
from concourse.tile import add_dep_helper

F32 = mybir.dt.float32
BF16 = mybir.dt.bfloat16
I16 = mybir.dt.int16
AX = mybir.AxisListType
OP = mybir.AluOpType
AF = mybir.ActivationFunctionType

N_CORES = 8
B, Q, T = 4, 256, 128
CELLS = (B * Q) // N_CORES  # 128 cells per core
GENE_VOCAB = 20000
GENE_DIM = 32
F = 64  # padded table row: [gs | gene_emb(32) | zeros], 256B for dma_gather
IN_DIM = 35
D = 256
NEG_SLOPE = 0.2
CHUNK_T = (48, 32, 24, 16, 8)  # big early, tiny tail round
NCH = len(CHUNK_T)
SINGLE_PACKET = False
assert sum(CHUNK_T) == T
NLATE = 2                # chunks NCH-NLATE.. land after the bulk matmul
TM = sum(CHUNK_T[:NCH - NLATE])  # early-transcript count (pooled_p x/qv)

# packed input column layout (f32 columns)
C_X = 0            # x: T*2
C_QV = C_X + T * 2       # 256
C_MADD = C_QV + T        # 384
C_CENT = C_MADD + T      # 512
C_ACT = C_CENT + 2       # 514
C_WAT = C_ACT + 1        # 515: wa01 tiled, 256
C_BG = C_WAT + 256       # 771: b_gat, 256
C_WA34 = C_BG + 256      # 1027
C_ID = C_WA34 + 1        # 1028: identity 128
C_WR = C_ID + 128        # 1156: wrhs bf16 [128, 256] as 128 f32
C_IDX = C_WR + 128       # 1284: idx int16 [128, 1024] as 512 f32
NB = C_IDX + (CELLS * T) // 32  # 1796

_CACHE = {}


def build_program():
    nc = bacc.Bacc("TRN2", target_bir_lowering=False, debug=False,
                   num_devices=N_CORES, num_swdge_queues=4)

    big_d = nc.dram_tensor("big_in", [CELLS, NB], F32, kind="ExternalInput")
    table_d = nc.dram_tensor("table_in", [GENE_VOCAB, F], F32, kind="ExternalInput")
    out_d = nc.dram_tensor("out", [CELLS, D], BF16, kind="ExternalOutput")

    from concourse import library_config

    with tile.TileContext(nc) as tc:
        with (
            tc.tile_pool(name="single", bufs=1) as single,
            tc.tile_pool(name="gpool", bufs=NCH) as gpool,
            tc.tile_pool(name="work", bufs=2) as work,
            tc.tile_pool(name="stats", bufs=NCH + 2) as stats,
            tc.tile_pool(name="psum", bufs=2, space="PSUM") as psum,
        ):
            # the mlp-library Q7 IRAM reload (~11us) gates the first gather;
            # it must be the first thing on the gpsimd stream
            nc.gpsimd.load_library(library_config.mlp)

            # one packed DMA for every dense input (sync HWDGE ring)
            big = single.tile([CELLS, NB], F32)
            nc.sync.dma_start(out=big[:], in_=big_d.ap())
            x_sb = big[:, C_X:C_X + T * 2]
            qv_sb = big[:, C_QV:C_QV + T]
            madd_sb = big[:, C_MADD:C_MADD + T]
            cent_sb = big[:, C_CENT:C_CENT + 2]
            act_sb = big[:, C_ACT:C_ACT + 1]
            wat_sb = big[:, C_WAT:C_WAT + 256]
            bg_sb = big[:, C_BG:C_BG + 256]
            wa34_sb = big[:, C_WA34:C_WA34 + 1]
            ident = big[:, C_ID:C_ID + 128]
            wrhs_sb = big[:IN_DIM, C_WR:C_WR + 128].bitcast(BF16)  # [35, 256]
            idx_sb = big[:, C_IDX:NB].bitcast(I16)  # [128, 1024] int16

            # --- gather pipeline (critical path) ---
            # queue pattern [1,2,3,0] per chunk: the three async queues
            # dispatch instantly, then the sync queue-0 op holds the engine
            # while all four queues' desc-gen runs concurrently. The
            # period-4 pattern also keeps Tile's 8 round-robin DMA-sem lanes
            # queue-consistent.
            g_t = []
            t_base = 0
            for j in range(NCH):
                tcnt = CHUNK_T[j]
                tq = tcnt // 4  # transcripts per quarter-gather
                gj = gpool.tile([CELLS, max(CHUNK_T) * F], F32, tag="g")
                g3w = gj[:].rearrange("p (t f) -> p t f", t=max(CHUNK_T), f=F)
                for sub, queue in enumerate((1, 2, 3, 0)):
                    c0 = (CELLS * (t_base + tq * sub)) // 16
                    nc.gpsimd.dma_gather(
                        out_ap=g3w[:, tq * sub:tq * (sub + 1), :],
                        in_ap=table_d.ap(),
                        idxs_ap=idx_sb[:, c0:c0 + (CELLS * tq) // 16],
                        num_idxs=CELLS * tq,
                        num_idxs_reg=CELLS * tq,
                        elem_size=F,
                        single_packet=SINGLE_PACKET,
                        queue_num=queue,
                    )
                g_t.append(gj)
                t_base += tcnt

            # b_gat * active, ready off the critical path for the epilogue
            bact = single.tile([CELLS, D], F32)
            nc.vector.tensor_scalar_mul(bact[:], bg_sb, act_sb)

            # --- s_pre = (rel . wa01) + qv*wa34 - cent.wa01 + madd  (no gene) ---
            xw = work.tile([CELLS, T * 2], F32)
            nc.vector.tensor_tensor(out=xw[:], in0=x_sb, in1=wat_sb, op=OP.mult)
            term_x = single.tile([CELLS, T], F32)
            nc.vector.tensor_reduce(
                out=term_x[:],
                in_=xw[:].rearrange("p (t c) -> p t c", t=T, c=2),
                axis=AX.X, op=OP.add)
            cw = stats.tile([CELLS, 2], F32, tag="cw")
            nc.vector.tensor_tensor(out=cw[:], in0=cent_sb, in1=wat_sb[:, 0:2],
                                    op=OP.mult)
            c01 = stats.tile([CELLS, 1], F32, tag="c01")
            nc.vector.tensor_reduce(out=c01[:], in_=cw[:], axis=AX.X, op=OP.add)
            negc01 = stats.tile([CELLS, 1], F32, tag="negc01")
            nc.vector.tensor_scalar_mul(negc01[:], c01[:], -1.0)
            qvs = work.tile([CELLS, T], F32, tag="qvs")
            nc.scalar.activation(out=qvs[:], in_=qv_sb, func=AF.Identity,
                                 bias=negc01[:], scale=wa34_sb)
            spre0 = work.tile([CELLS, T], F32, tag="spre0")
            nc.vector.tensor_tensor(out=spre0[:], in0=term_x[:], in1=qvs[:], op=OP.add)
            spre = single.tile([CELLS, T], F32)
            nc.vector.tensor_tensor(out=spre[:], in0=spre0[:], in1=madd_sb, op=OP.add)

            # --- per-chunk: s -> leaky relu -> exp -> weighted gene partials.
            # exp writes straight into e_all. Chunks 0..NCH-NLATE-1 feed
            # pooled_p -> an EARLY matmul into PSUM (start=True) that runs
            # under the late chunks' gathers; the NLATE late chunks feed
            # pooled_l, accumulated into the same bank after the last gather.
            e_all = single.tile([CELLS, T], F32, name="e_all", tag="e_all")
            pooled_p = single.tile([CELLS, IN_DIM], F32)  # early chunks
            pooled_l = single.tile([CELLS, IN_DIM], F32)  # late chunks + corr
            acc_es = None   # [CELLS, 1] running sum of e (early chunks)
            acc_pg = None   # [CELLS, 32] running gene pool (early chunks)
            late_es = []    # [CELLS, 1] per late chunk
            late_pg = []    # [CELLS, 32] per late chunk
            prev_pg_inst = None
            NEARLY = NCH - NLATE
            t_base = 0
            for j in range(NCH):
                tcnt = CHUNK_T[j]
                cj = slice(t_base, t_base + tcnt)
                g3 = g_t[j][:].rearrange("p (t f) -> p t f", t=max(CHUNK_T),
                                         f=F)[:, :tcnt, :]
                s_j = work.tile([CELLS, tcnt], F32, tag="s")
                s_inst = nc.vector.tensor_tensor(out=s_j[:], in0=spre[:, cj],
                                                 in1=g3[:, :, 0:1], op=OP.add)
                if prev_pg_inst is not None:
                    # keep the DVE stream in chunk order: without this the
                    # scheduler (using the serialized-gather cost model) parks
                    # every eg/pg op after the last chunk's s/l
                    add_dep_helper(s_inst.ins, prev_pg_inst.ins, False,
                                   "chunk-order DVE stream")
                l_j = work.tile([CELLS, tcnt], F32, tag="l")
                nc.vector.scalar_tensor_tensor(out=l_j[:], in0=s_j[:],
                                               scalar=NEG_SLOPE, in1=s_j[:],
                                               op0=OP.mult, op1=OP.max)
                esum_j = stats.tile([CELLS, 1], F32, tag="esum")
                nc.scalar.activation(out=e_all[:, cj], in_=l_j[:], func=AF.Exp,
                                     accum_out=esum_j[:])
                eg_j = work.tile([CELLS, tcnt * GENE_DIM], F32, tag="eg")
                nc.vector.tensor_tensor(
                    out=eg_j[:], in0=g3[:, :, 1:1 + GENE_DIM],
                    in1=e_all[:, cj].to_broadcast([CELLS, tcnt, GENE_DIM]),
                    op=OP.mult)
                pg_j = stats.tile([CELLS, GENE_DIM], F32, tag="pg")
                prev_pg_inst = nc.vector.tensor_reduce(
                    out=pg_j[:],
                    in_=eg_j[:].rearrange("p (t f) -> p f t", t=tcnt,
                                          f=GENE_DIM),
                    axis=AX.X, op=OP.add)
                if j >= NEARLY:
                    late_es.append(esum_j)
                    late_pg.append(pg_j)
                    if j == NEARLY:
                        # partial den = acc_es + es_late0 + 1, hidden under
                        # the last chunk's gather
                        es_l = stats.tile([CELLS, 1], F32, tag="esl")
                        nc.vector.tensor_scalar(es_l[:], acc_es[:],
                                                esum_j[:], 1.0, OP.add, OP.add)
                elif acc_es is None:
                    acc_es, acc_pg = esum_j, pg_j
                elif j < NEARLY - 1:
                    new_es = stats.tile([CELLS, 1], F32, tag="aes")
                    nc.vector.tensor_tensor(out=new_es[:], in0=acc_es[:],
                                            in1=esum_j[:], op=OP.add)
                    new_pg = stats.tile([CELLS, GENE_DIM], F32, tag="apg")
                    nc.vector.tensor_tensor(out=new_pg[:], in0=acc_pg[:],
                                            in1=pg_j[:], op=OP.add)
                    acc_es, acc_pg = new_es, new_pg
                else:
                    # final early accumulate writes straight into pooled_p
                    new_es = stats.tile([CELLS, 1], F32, tag="aes")
                    nc.vector.tensor_tensor(out=new_es[:], in0=acc_es[:],
                                            in1=esum_j[:], op=OP.add)
                    acc_es = new_es
                    nc.vector.tensor_tensor(out=pooled_p[:, 2:2 + GENE_DIM],
                                            in0=acc_pg[:], in1=pg_j[:],
                                            op=OP.add)
                t_base += tcnt
                if j == NEARLY - 1:
                    # batched pooled-x / pooled-qv over the early chunks,
                    # straight into pooled_p (runs under the late chunks'
                    # gathers), feeding the early half-matmul into PSUM
                    ex = work.tile([CELLS, TM * 2], F32, tag="exall")
                    nc.vector.tensor_tensor(
                        out=ex[:], in0=x_sb[:, :TM * 2],
                        in1=e_all[:, :TM].to_broadcast([CELLS, TM, 2]),
                        op=OP.mult)
                    nc.vector.tensor_reduce(
                        out=pooled_p[:, 0:2],
                        in_=ex[:].rearrange("p (t c) -> p c t", t=TM, c=2),
                        axis=AX.X, op=OP.add)
                    pqs_m = work.tile([CELLS, TM], F32, tag="pqm")
                    nc.vector.tensor_tensor(out=pqs_m[:], in0=qv_sb[:, :TM],
                                            in1=e_all[:, :TM], op=OP.mult)
                    nc.vector.tensor_reduce(out=pooled_p[:, 34:35],
                                            in_=pqs_m[:], axis=AX.X, op=OP.add)

            # early half-matmul into PSUM over the early chunks; emitted
            # after the loop so the ACT-queue copy cannot block the late
            # chunks' exp (engine queues are in emission order)
            psum_t1 = psum.tile([128, 128], F32, tag="pt")
            nc.tensor.transpose(out=psum_t1[:IN_DIM, :CELLS],
                                in_=pooled_p[:], identity=ident)
            lhsT1 = single.tile([128, CELLS], BF16)
            nc.scalar.copy(lhsT1[:IN_DIM, :], psum_t1[:IN_DIM, :CELLS])
            out_ps = psum.tile([128, D], F32, tag="out")
            nc.tensor.matmul(out=out_ps[:], lhsT=lhsT1[:IN_DIM, :],
                             rhs=wrhs_sb, start=True, stop=False)

            # --- late-chunk combine: pooled-x/qv (needs only e), gene pool ---
            TL = T - TM
            exl = stats.tile([CELLS, TL * 2], F32, tag="exl")
            nc.vector.tensor_tensor(
                out=exl[:], in0=x_sb[:, TM * 2:],
                in1=e_all[:, TM:].to_broadcast([CELLS, TL, 2]),
                op=OP.mult)
            pqs_l = stats.tile([CELLS, TL], F32, tag="pql")
            nc.vector.tensor_tensor(out=pqs_l[:], in0=qv_sb[:, TM:],
                                    in1=e_all[:, TM:], op=OP.mult)
            px_l = stats.tile([CELLS, 2], F32, tag="pxl")
            nc.vector.tensor_reduce(
                out=px_l[:], in_=exl[:].rearrange("p (t c) -> p c t", t=TL, c=2),
                axis=AX.X, op=OP.add)
            nc.vector.tensor_reduce(out=pooled_l[:, 34:35], in_=pqs_l[:],
                                    axis=AX.X, op=OP.add)
            nc.vector.tensor_tensor(out=pooled_l[:, 2:2 + GENE_DIM],
                                    in0=late_pg[0][:], in1=late_pg[1][:],
                                    op=OP.add)

            # den = (acc_es + es_late0 + 1) + es_late1
            den = stats.tile([CELLS, 1], F32, tag="den")
            nc.vector.tensor_tensor(out=den[:], in0=es_l[:],
                                    in1=late_es[1][:], op=OP.add)
            rec = stats.tile([CELLS, 1], F32, tag="rec")
            nc.vector.reciprocal(rec[:], den[:])
            ra = stats.tile([CELLS, 1], F32, tag="ra")
            nc.vector.tensor_scalar_mul(ra[:], rec[:], act_sb)

            # cs = cent * (den - 1) = cent*den - cent, one fused op;
            # pooled_l[0:2] = px_l - cs  (pooled stays UNNORMALIZED; 1/den
            # folds into the epilogue scale)
            cs = stats.tile([CELLS, 2], F32, tag="cs")
            nc.vector.scalar_tensor_tensor(out=cs[:], in0=cent_sb,
                                           scalar=den[:], in1=cent_sb,
                                           op0=OP.mult, op1=OP.subtract)
            nc.vector.tensor_tensor(out=pooled_l[:, 0:2], in0=px_l[:],
                                    in1=cs[:], op=OP.subtract)

            psum_t2 = psum.tile([128, 128], F32, tag="pt")
            nc.tensor.transpose(out=psum_t2[:IN_DIM, :CELLS], in_=pooled_l[:],
                                identity=ident)
            lhsT2 = single.tile([128, CELLS], BF16)
            nc.scalar.copy(lhsT2[:IN_DIM, :], psum_t2[:IN_DIM, :CELLS])
            nc.tensor.matmul(out=out_ps[:], lhsT=lhsT2[:IN_DIM, :],
                             rhs=wrhs_sb, start=False, stop=True)
            out_sb = work.tile([CELLS, D], BF16, tag="outs")
            nc.vector.scalar_tensor_tensor(out=out_sb[:], in0=out_ps[:],
                                           scalar=ra[:], in1=bact[:],
                                           op0=OP.mult, op1=OP.add)
            nc.sync.dma_start(out=out_d.ap(), in_=out_sb[:])

    nc.compile()
    return nc


def host_prep(omics_x, centroids, omics_gene_ids, omics_qv, omics_valid_mask,
              query_valid_mask, gene_emb, W_gat, a_src, a_dst, b_gat):
    import ml_dtypes
    f32 = np.float32
    wa = (W_gat.astype(np.float64) @ a_src.astype(np.float64)).astype(f32)  # [35]
    gs = (gene_emb.astype(f32) @ wa[2:2 + GENE_DIM]).astype(f32)  # [VOCAB]
    table = np.zeros((GENE_VOCAB, F), f32)  # [VOCAB, 64] (256B rows)
    table[:, 0] = gs
    table[:, 1:1 + GENE_DIM] = gene_emb.astype(f32)

    NC_TOT = B * Q
    x = omics_x.astype(f32).reshape(NC_TOT, T * 2)
    qv = omics_qv.astype(f32).reshape(NC_TOT, T)
    ids = omics_gene_ids.astype(np.int16).reshape(NC_TOT, T)
    cent = centroids.astype(f32).reshape(NC_TOT, 2)
    validf = omics_valid_mask.reshape(NC_TOT, T).astype(f32)
    madd = (validf - 1.0) * f32(1e30)
    active = (query_valid_mask.reshape(NC_TOT).astype(bool)
              & omics_valid_mask.reshape(NC_TOT, T).astype(bool).any(-1))
    active = active.astype(f32)

    wrhs_pad = np.zeros((128, D), f32)
    wrhs_pad[:IN_DIM] = W_gat.astype(f32)
    wrhs_bf = np.ascontiguousarray(
        wrhs_pad.astype(ml_dtypes.bfloat16)).view(np.uint16)  # [128, 256] u16

    in_maps = []
    for c in range(N_CORES):
        sl = slice(c * CELLS, (c + 1) * CELLS)
        big = np.zeros((CELLS, NB), f32)
        big[:, C_X:C_X + T * 2] = x[sl]
        big[:, C_QV:C_QV + T] = qv[sl]
        big[:, C_MADD:C_MADD + T] = madd[sl]
        big[:, C_CENT:C_CENT + 2] = cent[sl]
        big[:, C_ACT] = active[sl]
        big[:, C_WAT:C_WAT + 256] = np.tile(wa[0:2], T)[None, :]
        big[:, C_BG:C_BG + 256] = b_gat.astype(f32)[None, :]
        big[:, C_WA34] = wa[34]
        big[:, C_ID:C_ID + 128] = np.eye(128, dtype=f32)
        big[:, C_WR:C_WR + 128] = wrhs_bf.view(f32)
        # flat gather index i = t*CELLS + cell -> dst[cell, t]; wrapped
        # [128, T*CELLS/16] int16 (idx list tiled down the 8 core slabs)
        flat = ids[sl].T.reshape(-1)
        wrapped = np.ascontiguousarray(
            np.tile(flat.reshape(-1, 16).T, (8, 1)))  # [128, 1024] i16
        big[:, C_IDX:NB] = wrapped.reshape(128, -1).view(f32)
        in_maps.append({"big_in": np.ascontiguousarray(big), "table_in": table})
    return in_maps


def _get_program():
    # the program is fully parameter-independent: one compile, ever
    if "prog" not in _CACHE:
        _CACHE["prog"] = build_program()
    return _CACHE["prog"]


def kernel(omics_x, centroids, omics_gene_ids, omics_qv, omics_valid_mask,
           query_valid_mask, gene_emb, W_gat, a_src, a_dst, b_gat,
           trace=False):
    in_maps = host_prep(
        np.asarray(omics_x), np.asarray(centroids), np.asarray(omics_gene_ids),
        np.asarray(omics_qv), np.asarray(omics_valid_mask),
        np.asarray(query_valid_mask), np.asarray(gene_emb), np.asarray(W_gat),
        np.asarray(a_src), np.asarray(a_dst), np.asarray(b_gat))
    nc = _get_program()
    res = run_bass_kernel_spmd(nc, in_maps, core_ids=list(range(N_CORES)),
                               trace=trace)
    global LAST_RESULTS
    LAST_RESULTS = res
    outs = [np.asarray(res.results[c]["out"]).astype(np.float32)
            for c in range(N_CORES)]
    full = np.concatenate(outs, axis=0).reshape(B, Q, D)
    return full.astype(np.float32)
